# revision 53
# baseline (speedup 1.0000x reference)
"""Trainium2 Bass kernel for DifferentiableBiquadChain.

Per (batch, frame) lane the 16-biquad cascade is an LTI filter applied from
zero state to a 2048-sample frame.  The frame is split into 16 blocks of 128
samples.  Per core (2 batches = 256 lanes):

  within-block:  y_head = IDFT( DFT(x_block) o Hf_lane )        (circular
                 conv with the 128-tap impulse-response head, done as three
                 shared-weight matmul stages + an elementwise stage)
  cross-block:   32 slot states per lane (A/B components of the 16 pole
                 pairs) advance block-to-block via a Pool-engine scan; the
                 carry tail  y += sum_q beta_q S_q[r+1]  is one matmul per
                 4-lane group with a block-diagonal beta operand.

All matmul factors are fp16 (fp32 PSUM accumulation); per-pair rescaling
gamma = sqrt(|residue|) keeps carry factors in fp16 range.  Host precomputes
all parameter-derived tables in float64 (<0.5% of the FLOPs).
"""

import math
import sys

import numpy as np

sys.path.insert(0, "/opt/trn_rl_repo")

SR = 96000.0
FRAME = 2048
NB = 16
L = 128
NJ = 16
B_FULL, F = 16, 128
N = F * FRAME
N_CORES = 8
BPC = B_FULL // N_CORES          # batches per core = 2
NL = BPC * F                     # lanes per core = 256
NG = NL // 4                     # 4-lane groups = 64
CH = 8                           # column chunks of 512
GAIN_RANGE = (-24.0, 24.0)
Q_RANGE = (0.5, 16.0)
HPF_R = (20.0, 500.0)
LPF_R = (5000.0, 20000.0)
SHELF_R = (50.0, 16000.0)
PEAK_R = (100.0, 15000.0)
DMIN = 1e-8

# ---------------------------------------------------------------- host setup


def _denorm_freq(n, r):
    lo, hi = math.log(r[0]), math.log(r[1])
    return np.exp(lo + n * (hi - lo))


def _coeffs(params):
    B = params.shape[0]
    p = params.astype(np.float64)
    nl = B * F
    b0 = np.zeros((NB, nl)); b1 = np.zeros((NB, nl)); b2 = np.zeros((NB, nl))
    a1 = np.zeros((NB, nl)); a2 = np.zeros((NB, nl))
    for i in range(NB):
        fn = p[:, 3 * i, :].reshape(nl)
        gn = p[:, 3 * i + 1, :].reshape(nl)
        qn = p[:, 3 * i + 2, :].reshape(nl)
        Q = np.exp(math.log(Q_RANGE[0]) + qn * (math.log(Q_RANGE[1]) - math.log(Q_RANGE[0])))
        g = GAIN_RANGE[0] + gn * (GAIN_RANGE[1] - GAIN_RANGE[0])
        A = 10.0 ** (g / 40.0)
        if i == 0:
            fc, typ = _denorm_freq(fn, HPF_R), "hp"
        elif i == NB - 1:
            fc, typ = _denorm_freq(fn, LPF_R), "lp"
        elif i == 1:
            fc, typ = _denorm_freq(fn, SHELF_R), "ls"
        elif i == NB - 2:
            fc, typ = _denorm_freq(fn, SHELF_R), "hs"
        else:
            fc, typ = _denorm_freq(fn, PEAK_R), "pk"
        w0 = 2 * math.pi * fc / SR
        al = np.sin(w0) / (2 * Q)
        c = np.cos(w0)
        sA = np.sqrt(A)
        if typ == "hp":
            B0, B1, B2, A0, A1_, A2_ = (1 + c) / 2, -(1 + c), (1 + c) / 2, 1 + al, -2 * c, 1 - al
        elif typ == "lp":
            B0, B1, B2, A0, A1_, A2_ = (1 - c) / 2, 1 - c, (1 - c) / 2, 1 + al, -2 * c, 1 - al
        elif typ == "pk":
            B0, B1, B2, A0, A1_, A2_ = 1 + al * A, -2 * c, 1 - al * A, 1 + al / A, -2 * c, 1 - al / A
        elif typ == "ls":
            B0 = A * (A + 1 - (A - 1) * c + 2 * sA * al); B1 = 2 * A * (A - 1 - (A + 1) * c)
            B2 = A * (A + 1 - (A - 1) * c - 2 * sA * al)
            A0 = A + 1 + (A - 1) * c + 2 * sA * al; A1_ = -2 * (A - 1 + (A + 1) * c)
            A2_ = A + 1 + (A - 1) * c - 2 * sA * al
        else:
            B0 = A * (A + 1 + (A - 1) * c + 2 * sA * al); B1 = -2 * A * (A - 1 + (A + 1) * c)
            B2 = A * (A + 1 + (A - 1) * c - 2 * sA * al)
            A0 = A + 1 - (A - 1) * c + 2 * sA * al; A1_ = 2 * (A - 1 - (A + 1) * c)
            A2_ = A + 1 - (A - 1) * c - 2 * sA * al
        b0[i] = B0 / A0; b1[i] = B1 / A0; b2[i] = B2 / A0
        a1[i] = A1_ / A0; a2[i] = A2_ / A0
    n48 = p[:, 48, :].reshape(nl); n49 = p[:, 49, :].reshape(nl)
    gio = 10.0 ** (((-60.0 + n48 * 60.0) + (-60.0 + n49 * 60.0)) / 20.0)
    return b0, b1, b2, a1, a2, gio


def _pair_setup(b0, b1, b2, a1, a2, gio):
    disc = a1 * a1 / 4 - a2
    disc = np.where(np.abs(disc) > DMIN, disc, DMIN)
    s = np.sqrt(np.abs(disc))
    eps = np.sign(disc)
    h0 = -a1 / 2
    di = disc[:, None, :]
    wiA = (h0 / a2)[:, None, :]; wiB = (-1.0 / a2)[:, None, :]
    w2A = wiA * wiA + di * wiB * wiB
    w2B = 2 * wiA * wiB
    BA = b0[None] + b1[None] * wiA + b2[None] * w2A
    BB = b1[None] * wiB + b2[None] * w2B
    AA = 1.0 + a1[None] * wiA + a2[None] * w2A
    AB = a1[None] * wiB + a2[None] * w2B
    eye = np.eye(NB, dtype=bool)[:, :, None]
    AA = np.where(eye, 1.0, AA); AB = np.where(eye, 0.0, AB)
    n = AA * AA - di * AB * AB
    RA = (BA * AA - di * BB * AB) / n
    RB = (BB * AA - BA * AB) / n
    PA = RA[:, 0, :]; PB = RB[:, 0, :]
    for j in range(1, NB):
        PA, PB = (PA * RA[:, j] + disc * PB * RB[:, j], PA * RB[:, j] + PB * RA[:, j])
    dA = (a2 - h0 * h0 - disc) / a2; dB = 2 * h0 / a2
    nn = dA * dA - disc * dB * dB
    aA = (PA * dA - disc * PB * dB) / nn
    aB = (PB * dA - PA * dB) / nn
    cA = 2 * aA * gio
    cB = 2 * disc * aB / s * gio
    Dt = np.prod(b2, axis=0) / np.prod(a2, axis=0) * gio
    return h0, s, eps, cA, cB, Dt


def _slot_powers(h0, s, eps, n_max):
    sh = h0.shape
    SA = np.zeros(sh + (n_max + 1,)); SB = np.zeros_like(SA)
    SA[..., 0] = 1.0
    SA[..., 1] = h0; SB[..., 1] = s
    m = 1
    while m < n_max:
        t = min(m, n_max - m)
        mulA = SA[..., m:m + 1]; mulB = SB[..., m:m + 1]
        mulBe = eps[..., None] * mulB
        newA = SA[..., 1:1 + t] * mulA + SB[..., 1:1 + t] * mulBe
        newB = SA[..., 1:1 + t] * mulB + SB[..., 1:1 + t] * mulA
        SA[..., m + 1:m + 1 + t] = newA; SB[..., m + 1:m + 1 + t] = newB
        m *= 2
    return SA, SB


def _dft_mats():
    c_idx = np.arange(128)
    k_idx = np.arange(128)
    ang = 2 * np.pi * np.outer(c_idx, k_idx) / 256.0
    WA = np.cos(ang)
    WB = -np.sin(ang)
    WB[:, 0] = np.cos(np.pi * c_idx)
    angr = 2 * np.pi * np.outer(k_idx, c_idx) / 256.0
    wk = np.full((128, 1), 2.0); wk[0] = 1.0
    GA = wk * np.cos(angr) / 256.0
    GB = -2.0 * np.sin(angr) / 256.0
    GB[0] = np.cos(np.pi * c_idx) / 256.0
    return WA, WB, GA, GB


_WG_CACHE = {}


def host_tables(params_all):
    """Device-layout tables for ALL lanes at once (params (B,50,F)).  Lane
    l = b*128 + f; group g = l>>2, ls = l&3; slot q in [0,32): q=i for A of
    stage i, 16+i for B.  Column layouts are group-blocked, so the per-core
    slices are contiguous (global group = core*64 + local group)."""
    b0, b1, b2, a1, a2, gio = _coeffs(params_all)
    h0, s, eps, cA, cB, Dt = _pair_setup(b0, b1, b2, a1, a2, gio)
    SA, SB = _slot_powers(h0, s, eps, L)

    nl = params_all.shape[0] * F
    ng = nl // 4
    h = (cA[:, :, None] * SA[:, :, :L] + cB[:, :, None] * SB[:, :, :L]).sum(axis=0)
    h[:, 0] += Dt                                    # (nl, 128)
    Hf = np.fft.rfft(np.concatenate([h, np.zeros_like(h)], axis=1), axis=1)
    HRe, HIm = Hf.real, Hf.imag
    HA1 = HRe[:, :128].T.copy()
    HA2n = np.zeros((128, nl)); HA2n[1:] = -HIm[:, 1:128].T
    HB1 = np.zeros((128, nl)); HB1[1:] = HIm[:, 1:128].T
    HB2 = HRe[:, :128].T.copy(); HB2[0] = HRe[:, 128]
    # expanded over j, chunk-major cols (chunk, table, lane_in_chunk, j) so
    # the DVE pointwise gets 2x f16 packing; chunk = 32 consecutive lanes
    Hx = np.stack([HA1, HA2n, HB1, HB2], axis=1)          # [128, 4, nl]
    Hx = np.repeat(Hx[..., None], NJ, axis=3)             # [128, 4, nl, NJ]
    Hx = np.ascontiguousarray(
        Hx.reshape(128, 4, nl // 32, 32, NJ).transpose(0, 2, 1, 3, 4)
    ).reshape(128, 4 * nl * NJ).astype(np.float16)

    cmax = np.maximum(np.abs(cA), np.abs(cB))
    gam = np.sqrt(np.clip(cmax, 1e-4, 1e8))          # (NB, nl)
    Sq = np.concatenate([SA, SB], axis=0)            # (32, nl, 129)
    gam2 = np.concatenate([gam, gam], axis=0)
    pt = Sq[:, :, :L][:, :, ::-1] / gam2[:, :, None]     # [q, lane, c]
    zS = Sq[:, :, 1:L + 1] * gam2[:, :, None]            # [q, lane, r]
    ptP = np.ascontiguousarray(
        pt.reshape(32, ng, 4, L).transpose(3, 1, 2, 0).reshape(L, ng * 128)
    ).astype(np.float16)
    zPT = np.ascontiguousarray(
        zS.reshape(32, ng, 4, L).transpose(2, 0, 1, 3).reshape(L, ng * 128)
    ).astype(np.float16)

    sA128 = SA[:, :, L]; sB128 = SB[:, :, L]
    # scan/fold tables [128 rows=(ls,q), ng cols=g]
    def packT(valsA, valsB):
        # valsA for A rows (q<16), valsB for B rows: (NB, nl) each
        out = np.zeros((128, ng))
        va = valsA.reshape(NB, ng, 4)
        vb = valsB.reshape(NB, ng, 4)
        for ls in range(4):
            out[32 * ls:32 * ls + 16, :] = va[:, :, ls]
            out[32 * ls + 16:32 * ls + 32, :] = vb[:, :, ls]
        return out
    aA_t = packT(sA128, sA128)
    aBe_t = packT(eps * sB128, sB128)
    aBesw_t = packT(sB128, eps * sB128)
    # M^2 (256-sample advance) for the halved scan chain
    sA2 = sA128 * sA128 + eps * sB128 * sB128
    sB2 = 2.0 * sA128 * sB128
    aA2_t = packT(sA2, sA2)
    aBe2_t = packT(eps * sB2, sB2)
    aBesw2_t = packT(sB2, eps * sB2)
    cA_t = packT(cA, eps * cA)
    cB_t = packT(cB, cB)
    # composed odd-j fold: beta_odd = P.Z + Q.Zsw + cAt.w_even + cBt.wsw_even
    pa = cA * sA128 + cB * sB128
    qa = eps * cA * sB128 + cB * sA128
    P_t = packT(pa, eps * pa)
    Q_t = packT(qa, qa)
    # fused layouts for the [z|zs] state: aAx = [aA|aA], aBex = [aBe|aBesw]
    scTb = np.stack([aA_t, aA_t, aBe_t, aBesw_t,
                     aA2_t, aA2_t, aBe2_t, aBesw2_t,
                     cA_t, cB_t, P_t, Q_t]).astype(np.float32)  # [12,128,ng]

    if "wg" not in _WG_CACHE:
        WA, WB, GA, GB = _dft_mats()
        _WG_CACHE["wg"] = np.concatenate([WA, WB, GA, GB], axis=1).astype(np.float16)
    return ptP, zPT, _WG_CACHE["wg"], Hx, scTb


def _core_tables(tabs, k):
    """Per-core input dict (minus X) sliced out of the full-width tables."""
    ptP, zPT, WG, Hx, scTb = tabs
    gsl = slice(k * NG * 128, (k + 1) * NG * 128)
    # Hx is chunk-major over all lanes; a core's 8 chunks are contiguous
    hsl = slice(k * 4 * NL * NJ, (k + 1) * 4 * NL * NJ)
    scT = np.ascontiguousarray(
        scTb[:, :, k * NG:(k + 1) * NG].transpose(1, 0, 2)).reshape(128, 768)
    return {
        "ptP": np.ascontiguousarray(ptP[:, gsl]),
        "zPT": np.ascontiguousarray(zPT[:, gsl]),
        "WG": WG, "Hx": np.ascontiguousarray(Hx[:, hsl]), "scT": scT,
    }


# ---------------------------------------------------------------- device code

_prog_cache = {}


def _build_program(split_waits=True):
    import concourse.bass as bass
    import concourse.tile as tile
    import concourse.mybir as mb
    import bass_rust

    f16 = mb.dt.float16
    f32 = mb.dt.float32
    Alu = mb.AluOpType
    nc = bass.Bass("TRN2", target_bir_lowering=False, debug=False)

    X_d = nc.dram_tensor("X", [128, NL * NJ], f16, kind="ExternalInput").ap()
    ptP_d = nc.dram_tensor("ptP", [128, NG * 128], f16, kind="ExternalInput").ap()
    zPT_d = nc.dram_tensor("zPT", [128, NG * 128], f16, kind="ExternalInput").ap()
    WG_d = nc.dram_tensor("WG", [128, 512], f16, kind="ExternalInput").ap()
    Hx_d = nc.dram_tensor("Hx", [128, 4 * NL * NJ], f16, kind="ExternalInput").ap()
    scT_d = nc.dram_tensor("scT", [128, 768], f32, kind="ExternalInput").ap()
    y_d = nc.dram_tensor("y", [128, NL * NJ], f16, kind="ExternalOutput").ap()

    with tile.TileContext(nc) as tc:
        with tc.tile_pool(name="tab", bufs=1) as tab, \
             tc.tile_pool(name="wk", bufs=1) as wk, \
             tc.tile_pool(name="tch", bufs=8) as tch, \
             tc.tile_pool(name="psW", bufs=2, space="PSUM") as psWp, \
             tc.tile_pool(name="psAB", bufs=1, space="PSUM") as psABp, \
             tc.tile_pool(name="psY", bufs=4, space="PSUM") as psYp:

            X = tab.tile([128, NL * NJ], f16, name="X")
            ptPs = tab.tile([128, NG * 128], f16, name="ptPs")
            zPTs = tab.tile([128, NG * 128], f16, name="zPTs")
            WG = tab.tile([128, 512], f16, name="WG")
            Hx = tab.tile([128, 4 * NL * NJ], f16, name="Hx")
            scT = tab.tile([128, 768], f32, name="scT")
            # DMA order: Wend needs ptP+X first; zPT only needed by the late
            # carry matmuls.  Split the early ones so Wend starts sooner.
            QP = NG * 128 // 4
            QX = NL * NJ // 4
            for q in range(4):
                nc.sync.dma_start(ptPs[:, q * QP:(q + 1) * QP],
                                  ptP_d[:, q * QP:(q + 1) * QP])
                nc.sync.dma_start(X[:, q * QX:(q + 1) * QX],
                                  X_d[:, q * QX:(q + 1) * QX])
            nc.sync.dma_start(scT[:], scT_d[:, :])
            nc.sync.dma_start(WG[:], WG_d[:, :])
            HXW = 2 * NL * NJ
            nc.sync.dma_start(Hx[:, 0:HXW], Hx_d[:, 0:HXW])
            nc.sync.dma_start(Hx[:, HXW:2 * HXW], Hx_d[:, HXW:2 * HXW])
            nc.sync.dma_start(zPTs[:], zPT_d[:, :])

            # ---- Wend: one grouped matmul per 4-lane group; out[(ls',q),
            # (ls,j)] is valid on the ls'=ls diagonal blocks, which the
            # per-batch eviction extracts.  4x redundant PE compute but 4x
            # fewer PE instructions than per-lane matmuls.
            wboth = wk.tile([128, 2 * NG * NJ], f32, name="wboth")
            W1 = NG * NJ
            NGB = 8                        # groups per psum batch (1 bank)
            for b in range(NG // NGB):
                psW = psWp.tile([128, NGB * 64], f32, name="psW")
                for gi in range(NGB):
                    g = b * NGB + gi
                    nc.tensor.matmul(
                        psW[:, gi * 64:(gi + 1) * 64],
                        ptPs[:, g * 128:(g + 1) * 128],
                        X[:, g * 64:(g + 1) * 64],
                        start=True, stop=True, skip_group_check=True)
                for ls in range(4):
                    src = psW[32 * ls:32 * ls + 32, :].rearrange(
                        "p (gi l j) -> p gi l j", l=4, j=NJ)[:, :, ls, :]
                    dst = wboth[32 * ls:32 * ls + 32,
                                b * NGB * NJ:(b + 1) * NGB * NJ].rearrange(
                        "p (gi j) -> p gi j", j=NJ)
                    nc.scalar.copy(dst, src)
            # wsw = A<->B partition swap (DMA)
            for ls in range(4):
                for hh in range(2):
                    nc.sync.dma_start(
                        wboth[32 * ls + 16 * (1 - hh):32 * ls + 16 * (1 - hh) + 16,
                              W1:2 * W1],
                        wboth[32 * ls + 16 * hh:32 * ls + 16 * hh + 16, 0:W1])

            # ---- M^2 scan: Z_k = z_{2k} advances 7 serial steps on Pool;
            # pair-combine (pre) and odd-state fill (post) are wide DVE ops.
            NK = NJ // 2
            aA2q = scT[:, 256:384].rearrange("p (h g) -> p h g", h=2)
            aBe2q = scT[:, 384:512].rearrange("p (h g) -> p h g", h=2)
            cAt = scT[:, 512:576]
            cBt = scT[:, 576:640]
            def hgview(tile, cols, off, hstride, gstride, blk=None, bstride=None):
                v = tile[:].copy()
                dims = [[v.ap[0][0], 128]]
                if blk is not None:
                    dims.append([bstride, blk])
                dims += [[hstride, 2], [gstride, NG]]
                v.ap = bass_rust.VecI64Pair(dims)
                v.offset = v.offset + off
                return v

            # pre: v_k = M.w_{2k} + w_{2k+1}   (k = 0..7)  [128,(k,h,g)]
            vt = wk.tile([128, NK * 128], f32, name="vt")
            pt1 = wk.tile([128, NK * 128], f32, name="pt1")
            Wev = hgview(wboth, None, 0, W1, NJ, blk=NK, bstride=2)
            WevSW = hgview(wboth, None, W1, -W1, NJ, blk=NK, bstride=2)
            Wod = hgview(wboth, None, 1, W1, NJ, blk=NK, bstride=2)
            vt4 = vt[:].rearrange("p (k h g) -> p k h g", k=NK, h=2)
            pt4 = pt1[:].rearrange("p (k h g) -> p k h g", k=NK, h=2)
            aA1b4 = scT[:, 0:128].unsqueeze(1).broadcast_to(
                [128, NK, 128]).rearrange("p k (h g) -> p k h g", h=2)
            aBe1b4 = scT[:, 128:256].unsqueeze(1).broadcast_to(
                [128, NK, 128]).rearrange("p k (h g) -> p k h g", h=2)
            nc.vector.tensor_tensor(vt4, aA1b4, Wev, op=Alu.mult)
            nc.vector.tensor_tensor(pt4, aBe1b4, WevSW, op=Alu.mult)
            nc.vector.tensor_tensor(vt[:], vt[:], pt1[:], op=Alu.add)
            nc.vector.tensor_tensor(vt4, vt4, Wod, op=Alu.add)

            # chain on Pool: Z_0 = 0; Z_k = M^2 Z_{k-1} + v_{k-1}
            Zb = wk.tile([128, NK * 128], f32, name="Zb")
            st1 = wk.tile([128, 128], f32, name="st1")
            st2 = wk.tile([128, 128], f32, name="st2")
            bpad_early = wk.tile([128, NG * 4 * NJ], f16, name="bpad")
            nc.gpsimd.memset(bpad_early[:], 0.0)
            nc.gpsimd.memset(Zb[:, 0:128], 0.0)
            st1v = st1[:].rearrange("p (h g) -> p h g", h=2)
            st2v = st2[:].rearrange("p (h g) -> p h g", h=2)
            for k in range(1, NK):
                Zp = Zb[:, (k - 1) * 128:k * 128].rearrange("p (h g) -> p h g", h=2)
                Zpsw = hgview(Zb, None, (k - 1) * 128 + 64, -64, 1)
                vk = vt[:, (k - 1) * 128:k * 128].rearrange("p (h g) -> p h g", h=2)
                Zn = Zb[:, k * 128:(k + 1) * 128].rearrange("p (h g) -> p h g", h=2)
                nc.gpsimd.tensor_tensor(st1v, aA2q, Zp, op=Alu.mult)
                nc.gpsimd.tensor_tensor(st2v, aBe2q, Zpsw, op=Alu.mult)
                nc.gpsimd.tensor_tensor(st1v, st1v, st2v, op=Alu.add)
                nc.gpsimd.tensor_tensor(Zn, st1v, vk, op=Alu.add)

            # ---- chunk machinery.  part A: DFT (PE) + 4 H-mults (DVE /
            # Pool for ch 7).  part B: 4-MM IDFT + 8 carry MMs + evict.
            # PE is in-order, so emission interleaves DFT/IDFT by readiness.
            bpad = bpad_early
            yob = wk.tile([128, NL * NJ], f16, name="yob")
            tts = {}
            ys = {}

            def chunk_a(ch, eng):
                xc = X[:, ch * 512:(ch + 1) * 512]
                AB = psABp.tile([128, 1024], f32, name="AB")
                nc.tensor.matmul(AB[:, 0:512], WG[:, 0:128], xc,
                                 start=True, stop=True, skip_group_check=True)
                nc.tensor.matmul(AB[:, 512:1024], WG[:, 128:256], xc,
                                 start=True, stop=True, skip_group_check=True)
                abf = tch.tile([128, 1024], f16, name="abf")
                nc.scalar.copy(abf[:], AB[:])
                tt = tch.tile([128, 2048], f16, name="tt")
                hc = ch * 2048
                for i, pcol in enumerate((0, 512, 0, 512)):
                    eng.tensor_tensor(
                        tt[:, i * 512:(i + 1) * 512],
                        abf[:, pcol:pcol + 512],
                        Hx[:, hc + i * 512:hc + (i + 1) * 512], op=Alu.mult)
                tts[ch] = tt

            def idft(ch):
                tt = tts[ch]
                Y = psYp.tile([128, 512], f32, name="Y")
                for i, wcol in enumerate((256, 256, 384, 384)):
                    nc.tensor.matmul(Y[:], WG[:, wcol:wcol + 128],
                                     tt[:, i * 512:(i + 1) * 512],
                                     start=(i == 0), stop=False,
                                     skip_group_check=True)
                ys[ch] = Y

            def carry_evict(ch):
                Y = ys[ch]
                for gi in range(8):
                    g = ch * 8 + gi
                    nc.tensor.matmul(Y[:, gi * 64:(gi + 1) * 64],
                                     zPTs[:, g * 128:(g + 1) * 128],
                                     bpad[:, g * 64:(g + 1) * 64],
                                     start=False, stop=(gi == 7),
                                     skip_group_check=True)
                ysl = yob[:, ch * 512:(ch + 1) * 512]
                if ch < 4:
                    nc.scalar.copy(ysl, Y[:])       # Act
                else:
                    nc.vector.tensor_scalar_add(ysl, Y[:], 0.0)  # DVE (idle late)
                if ch % 2 == 1:                      # quarter-sized output DMAs
                    nc.sync.dma_start(y_d[:, (ch - 1) * 512:(ch + 1) * 512],
                                      yob[:, (ch - 1) * 512:(ch + 1) * 512])

            for ch in range(3):
                chunk_a(ch, nc.vector)

            # ---- fold -> beta (even j direct from Zb on Pool; odd j via
            # host-composed tables on DVE: beta_odd = P.Z + Q.Zsw
            #                                        + cAt.w_ev + cBt.wsw_ev)
            bfull = wk.tile([128, NJ * NG], f16, name="bfull")
            fe1 = wk.tile([128, NK * NG], f32, name="fe1")
            fe2 = wk.tile([128, NK * NG], f32, name="fe2")
            fo1 = wk.tile([128, NK * NG], f32, name="fo1")
            fo2 = wk.tile([128, NK * NG], f32, name="fo2")
            cab = cAt.unsqueeze(1).broadcast_to([128, NK, NG])
            cbb = cBt.unsqueeze(1).broadcast_to([128, NK, NG])
            Pb = scT[:, 640:704].unsqueeze(1).broadcast_to([128, NK, NG])
            Qb = scT[:, 704:768].unsqueeze(1).broadcast_to([128, NK, NG])
            Zr = Zb[:].rearrange("p (k h g) -> p k h g", k=NK, h=2)
            Zh0 = Zr[:, :, 0, :]
            Zh1 = Zr[:, :, 1, :]
            Wh0e = wboth[:].copy()
            Wh0e.ap = bass_rust.VecI64Pair([[Wh0e.ap[0][0], 128], [2, NK], [NJ, NG]])
            Wh1e = wboth[:].copy()
            Wh1e.ap = bass_rust.VecI64Pair([[Wh1e.ap[0][0], 128], [2, NK], [NJ, NG]])
            Wh1e.offset = Wh1e.offset + W1
            f1v = fe1[:].rearrange("p (k g) -> p k g", k=NK)
            f2v = fe2[:].rearrange("p (k g) -> p k g", k=NK)
            o1v = fo1[:].rearrange("p (k g) -> p k g", k=NK)
            o2v = fo2[:].rearrange("p (k g) -> p k g", k=NK)
            bev = bfull[:].rearrange("p (j g) -> p j g", j=NJ)[:, 0::2, :]
            bod = bfull[:].rearrange("p (j g) -> p j g", j=NJ)[:, 1::2, :]
            # even j (Pool, after chain)
            nc.gpsimd.tensor_tensor(f1v, cab, Zh0, op=Alu.mult)
            nc.gpsimd.tensor_tensor(f2v, cbb, Zh1, op=Alu.mult)
            nc.gpsimd.tensor_tensor(bev, f1v, f2v, op=Alu.add)
            # odd j (DVE)
            nc.vector.tensor_tensor(o1v, Pb, Zh0, op=Alu.mult)
            nc.vector.tensor_tensor(o2v, Qb, Zh1, op=Alu.mult)
            nc.vector.tensor_tensor(fo1[:], fo1[:], fo2[:], op=Alu.add)
            nc.vector.tensor_tensor(o2v, cab, Wh0e, op=Alu.mult)
            nc.vector.tensor_tensor(fo1[:], fo1[:], fo2[:], op=Alu.add)
            nc.vector.tensor_tensor(o2v, cbb, Wh1e, op=Alu.mult)
            nc.vector.tensor_tensor(bod, o1v, o2v, op=Alu.add)

            # ---- beta_pad scatter: block-diagonal [128, (g, ls, j)]
            bp4 = bpad[:].rearrange("p (g l j) -> p g l j", l=4, j=NJ)
            for ls in range(4):
                src = bfull[32 * ls:32 * ls + 32, :].rearrange(
                    "p (j g) -> p g j", j=NJ)
                dst = bp4[32 * ls:32 * ls + 32, :, ls, :]
                if ls < 2:
                    nc.scalar.copy(dst, src)
                elif ls == 2:
                    nc.vector.tensor_scalar_add(dst, src, 0.0)
                else:
                    nc.gpsimd.tensor_scalar_add(dst, src, 0.0)

            # readiness-ordered PE stream: remaining DFTs interleaved with
            # the first four IDFTs, then carries, then the late IDFT+carry
            # pairs as PSUM banks free up.
            idft(0)
            chunk_a(3, nc.vector)
            idft(1)
            chunk_a(4, nc.vector)
            idft(2)
            chunk_a(5, nc.vector)
            idft(3)
            chunk_a(6, nc.vector)
            chunk_a(7, nc.vector)
            for ch in range(4):
                carry_evict(ch)
            for ch in range(4, CH):
                idft(ch)
                carry_evict(ch)

    # walrus rejects >1 sync-wait per instruction on this toolchain
    if not split_waits:
        return nc
    import concourse.mybir as mb2
    fn = nc.m.functions[0]
    for bb in fn.blocks:
        insts = bb.instructions
        i = 0
        while i < len(insts):
            inst = insts[i]
            si = inst.sync_info
            if si is not None and si.on_wait and len(si.on_wait) > 1:
                waits = list(si.on_wait)
                extra, keep = waits[:-1], waits[-1:]
                new_nops = []
                for k, w in enumerate(extra):
                    nop = mb2.InstNoOp(name=f"{inst.name}_wsplit{k}", ins=[], outs=[])
                    nop.engine = inst.engine
                    nop.sync_info = mb2.SyncInfo(on_wait=[w], on_update=[])
                    new_nops.append(nop)
                si.on_wait = keep
                insts[i:i] = new_nops
                i += len(new_nops)
            i += 1
    return nc


def _prep_core_inputs(audio_core, params_core):
    """Single-core input dict (used by the CoreSim harness)."""
    tabs = host_tables(params_core)
    m = _core_tables(tabs, 0)
    xr = audio_core.reshape(BPC, F, NJ, L)
    m["X"] = np.ascontiguousarray(
        xr.transpose(3, 0, 1, 2).reshape(128, NL * NJ)).astype(np.float16)
    return m


def _prep_x(audio):
    """Concatenated X for all cores: [8*128, 4096] f16."""
    xr = audio.reshape(N_CORES, BPC, F, NJ, L)
    return np.ascontiguousarray(
        xr.transpose(0, 4, 1, 2, 3).reshape(N_CORES * 128, NL * NJ)
    ).astype(np.float16)


def _get_runner():
    """Persistent jitted SPMD executor (trace once, reuse across calls)."""
    if "runner" in _prog_cache:
        return _prog_cache["runner"]
    import jax
    import jax.numpy as jnp
    import concourse.mybir as mybir
    from concourse.bass2jax import (_bass_exec_p, install_neuronx_cc_hook,
                                    partition_id_tensor)
    from jax.sharding import Mesh, PartitionSpec, NamedSharding
    from jax.experimental.shard_map import shard_map

    install_neuronx_cc_hook()
    nc = _prog_cache["nc"]
    part_name = (nc.partition_id_tensor.name
                 if nc.partition_id_tensor is not None else None)
    in_names, out_names, out_avals = [], [], []
    for alloc in nc.m.functions[0].allocations:
        if not isinstance(alloc, mybir.MemoryLocationSet):
            continue
        name = alloc.memorylocations[0].name
        if alloc.kind == "ExternalInput":
            if name != part_name:
                in_names.append(name)
        elif alloc.kind == "ExternalOutput":
            out_names.append(name)
            out_avals.append(jax.core.ShapedArray(
                tuple(alloc.tensor_shape), mybir.dt.np(alloc.dtype)))
    n_params = len(in_names)
    all_names = in_names + out_names
    if part_name is not None:
        all_names = all_names + [part_name]
    donate = tuple(range(n_params, n_params + len(out_names)))

    def _body(*args):
        operands = list(args)
        if part_name is not None:
            operands.append(partition_id_tensor())
        outs = _bass_exec_p.bind(
            *operands,
            out_avals=tuple(out_avals),
            in_names=tuple(all_names),
            out_names=tuple(out_names),
            lowering_input_output_aliases=(),
            sim_require_finite=True,
            sim_require_nnan=True,
            nc=nc,
        )
        return tuple(outs)

    devices = jax.devices()[:N_CORES]
    mesh = Mesh(np.asarray(devices), ("core",))
    sh = NamedSharding(mesh, PartitionSpec("core"))
    n_in = n_params + len(out_names)
    sharded = jax.jit(
        shard_map(_body, mesh=mesh,
                  in_specs=(PartitionSpec("core"),) * n_in,
                  out_specs=(PartitionSpec("core"),) * len(out_names),
                  check_rep=False),
        donate_argnums=donate, keep_unused=True)
    zshapes = [(N_CORES * a.shape[0], *a.shape[1:]) for a in out_avals]
    zdtypes = [a.dtype for a in out_avals]
    mkzeros = jax.jit(
        lambda: tuple(jnp.zeros(s, d) for s, d in zip(zshapes, zdtypes)),
        out_shardings=(sh,) * len(out_names))
    runner = {"fn": sharded, "mkzeros": mkzeros, "sh": sh,
              "in_names": in_names, "jax": jax}
    _prog_cache["runner"] = runner
    return runner


def kernel(audio, params):
    audio = np.asarray(audio, dtype=np.float32)
    params = np.asarray(params, dtype=np.float32)
    if "nc" not in _prog_cache:
        _prog_cache["nc"] = _build_program()
    r = _get_runner()
    jax = r["jax"]

    pkey = hash(params.tobytes())
    if _prog_cache.get("pkey") != pkey:
        tabs = host_tables(params)
        per_core = [_core_tables(tabs, k) for k in range(N_CORES)]
        dev_tabs = {}
        for name in r["in_names"]:
            if name == "X":
                continue
            cat = np.concatenate([per_core[k][name] for k in range(N_CORES)],
                                 axis=0)
            dev_tabs[name] = jax.device_put(cat, r["sh"])
        _prog_cache["pkey"] = pkey
        _prog_cache["dev_tabs"] = dev_tabs
    dev_tabs = _prog_cache["dev_tabs"]

    Xc = jax.device_put(_prep_x(audio), r["sh"])
    args = [Xc if n == "X" else dev_tabs[n] for n in r["in_names"]]
    # donated output buffers: recycle the previous call's outputs (every
    # output element is written by the kernel, so contents don't matter)
    zeros = _prog_cache.pop("prev_outs", None)
    if zeros is None:
        zeros = r["mkzeros"]()
    outs = r["fn"](*args, *zeros)
    yr = np.asarray(outs[0]).astype(np.float32)     # [8*128, 4096] f16
    _prog_cache["prev_outs"] = outs
    yl = yr.reshape(N_CORES, 128, NL, NJ).transpose(0, 2, 3, 1)
    return yl.reshape(N_CORES * BPC, F, NJ * L).reshape(B_FULL, N)


if __name__ == "__main__":
    rng = np.random.default_rng(0)
    a = rng.standard_normal((B_FULL, N)).astype(np.float32)
    p = rng.random((B_FULL, 50, F)).astype(np.float32)
    y = kernel(a, p)
    print(y.shape, np.abs(y).max())
